# revision 19
# baseline (speedup 1.0000x reference)
"""Trainium2 Bass kernel for the AnnRC spiking-RNN problem.

Strategy: pure data parallelism across batch (8 cores x 32 rows each).

Input matmul (C = 0.5*(x @ W_in + bias), all frames), computed at PSUM
scale 2^22 so three passes share one accumulation group:
  p1: fp16(x*2^11) @ fp16(W*2^11)            (exact products)
  p2: e4m3(xl*2^13) @ e4m3(W*2^9)   fp8 DoubleRow, 0.5 cyc/row
  p3: e4m3(x*2^5)   @ e4m3(Wl*2^17) fp8 DoubleRow, 0.5 cyc/row
(xl = x - fp16(x), Wl = W - fp16(W).)  The 2^-22 unscale rides the
PSUM->SBUF copy (tensor_scalar mult).  numpy sim (sim_precision.py):
319 spike flips vs the ~476-flip budget (rel-err gate 2e-2); the sim
reproduces hw exactly (baseline scheme 43 sim vs 40 hw, fp16-A 1963 vs
1964).  DoubleRow pairs two k-tiles per instruction ([128,2,f] APs, both
operands fp8e4) and is only legal WITHOUT tile_position col-tiling,
which is why it applies to the U passes (M=128) and not the recurrence.

Recurrence per frame:
  r' = spike @ (0.5*A)  -- A as an exact bf16 hi+lo split, two
       accumulating passes (single-pass fp16/bf16 A fails: ~1964 flips).
       The four 512-wide output chunks run CONCURRENTLY in the four
       column groups of the PE array via tile_position (M=32 each) --
       measured 216ns per 4x512-row quartet = full streaming roofline.
  Per 128-wide block j (pipelined):  t = r' + C_t (DVE add from PSUM);
  full 128x128 PE transpose into packed hidden-major order; y = tanh
  (ACT, reading PSUM);  spike' = (y > 0.5 - h)  (DVE is_gt, bf16 out ->
  feeds the next frame's stationary directly).
  Off the critical chain: mem' = y + h, h' = 0.5*mem' + 0.5*spike' - 0.5.

DMA: bulk weights (Wh/Wh8/Wl8, A hi/lo) issue on the SCALAR engine's
queue; streaming traffic (x chunks, C roundtrip, outputs) stays on SYNC
so frame 0's C read is not stuck behind 33MB of weight loads (the v1
prologue idled the PE for ~70us this way).  A DMAs are emitted after
the two PRE U-chunks so the first matmuls start as soon as W lands.

State is hidden-major packed [128, (j, hc, b)] so elementwise ops use all
128 partitions and spike slices feed the matmul stationary without any
per-frame transposition. Outputs are written packed and unpacked on host.
"""

import os
import sys
import types

import numpy as np

# ---------------------------------------------------------------------------
# antenv.axon_hooks shim: this image's antenv lacks the module, and
# concourse.bass_utils imports it unconditionally when tracing is requested.
if "antenv.axon_hooks" not in sys.modules:
    _hooks_mod = types.ModuleType("antenv.axon_hooks")
    _hooks_mod._hook = None
    _hooks_mod.set_axon_ntff_profile_hook = lambda h: setattr(_hooks_mod, "_hook", h)
    _hooks_mod.get_axon_ntff_profile_hook = lambda: _hooks_mod._hook
    sys.modules["antenv.axon_hooks"] = _hooks_mod
    try:
        from trn_agent_boot.trn_boot import _ntff_profile_via_ctypes

        _hooks_mod._hook = _ntff_profile_via_ctypes("/opt/axon/libaxon_pjrt.so")
    except Exception:
        pass

import concourse.bacc as bacc
import concourse.bass_utils as bass_utils
import concourse.mybir as mybir
import concourse.tile as tile
from concourse.bass_utils import run_bass_kernel_spmd

# Zero-egress container: artifact upload would fail; keep local.
bass_utils.upload_artifacts = lambda tmpdir: tmpdir

ALPHA, DECAY, THR = 0.5, 0.5, 0.5
N_IN, N_HID = 700, 2048
BATCH, FRAMES = 256, 128
NCORES = 8
B = BATCH // NCORES          # 32 batch rows per core
KT = N_HID // 128            # 16 k-tiles of the recurrent contraction
HC = N_HID // 512            # 4 n-chunks of 512
KIN = 768                    # padded input contraction (700 + 1 bias + pad)
KC = KIN // 128              # 6 k-tiles for the input matmul
BT = B * FRAMES              # 4096 (batch,frame) pairs per core
BTC = BT // 128              # 32 chunks of 128 bt-pairs

F32 = mybir.dt.float32
BF16 = mybir.dt.bfloat16
FP16 = mybir.dt.float16
F8E4 = mybir.dt.float8e4
DR = mybir.MatmulPerfMode.DoubleRow

USC = float(2.0 ** -22)      # PSUM unscale for the input matmul

LAST_RESULT = None  # test.py reads .exec_time_ns off this after a traced call

_NC_CACHE = {}


def _build_nc(frames):
    nc = bacc.Bacc("TRN2", target_bir_lowering=False, debug=False, num_devices=NCORES)

    Aph = nc.declare_dram_parameter("Aph", [128, KT * N_HID], BF16, isOutput=False)
    Apl = nc.declare_dram_parameter("Apl", [128, KT * N_HID], BF16, isOutput=False)
    Wph = nc.declare_dram_parameter("Wph", [128, KC * N_HID], FP16, isOutput=False)
    Wp8h = nc.declare_dram_parameter("Wp8h", [128, KC, N_HID], F8E4, isOutput=False)
    Wp8l = nc.declare_dram_parameter("Wp8l", [128, KC, N_HID], F8E4, isOutput=False)
    xTh = nc.declare_dram_parameter("xTh", [KIN, BT], FP16, isOutput=False)
    xT8h = nc.declare_dram_parameter("xT8h", [KIN, BT], F8E4, isOutput=False)
    xT8l = nc.declare_dram_parameter("xT8l", [KIN, BT], F8E4, isOutput=False)
    mem0 = nc.declare_dram_parameter("mem0", [128, 512], F32, isOutput=False)
    eye = nc.declare_dram_parameter("eye", [128, 128], F32, isOutput=False)
    memsT = nc.declare_dram_parameter("memsT", [frames, 128, 512], F32, isOutput=True)
    spikesT = nc.declare_dram_parameter("spikesT", [frames, 128, 512], BF16, isOutput=True)

    btc_used = (B * frames + 127) // 128

    with tile.TileContext(nc) as tc:
        with (
            tc.tile_pool(name="dram", bufs=1, space="DRAM") as dram,
            tc.tile_pool(name="state", bufs=1) as st,
            tc.tile_pool(name="big", bufs=1) as big,
        ):
            C_d = dram.tile([FRAMES, 128, 512], F32, tag="C")
            eye_sb = st.tile([128, 128], F32, tag="eye")
            nc.scalar.dma_start(eye_sb[:], eye[:])
            mem0_sb = st.tile([128, 512], F32, tag="mem0")
            nc.scalar.dma_start(mem0_sb[:], mem0[:])

            # W first (per k-tile so the first matmuls start after ~2MB);
            # A is emitted AFTER the PRE U-chunks below, and all bulk
            # weights ride the scalar engine's DMA queue so the sync
            # queue (x chunks, C roundtrip, outputs) is never blocked.
            Wh_sb = big.tile([128, KC * N_HID], FP16, tag="Wh")
            W8h_sb = big.tile([128, KC, N_HID], F8E4, tag="W8h")
            W8l_sb = big.tile([128, KC, N_HID], F8E4, tag="W8l")
            for kc in range(KC):
                sl = slice(kc * N_HID, (kc + 1) * N_HID)
                nc.scalar.dma_start(Wh_sb[:, sl], Wph[:, sl])
                nc.scalar.dma_start(W8h_sb[:, kc, :], Wp8h[:, kc, :])
                nc.scalar.dma_start(W8l_sb[:, kc, :], Wp8l[:, kc, :])
            A_hi = big.tile([128, KT * N_HID], BF16, tag="Ahi")
            A_lo = big.tile([128, KT * N_HID], BF16, tag="Alo")

            with (
                tc.tile_pool(name="ustg", bufs=2) as ustg,
                tc.tile_pool(name="ups", bufs=4, space="PSUM") as ups,
                tc.tile_pool(name="fstg", bufs=2) as fs,
                tc.tile_pool(name="cpool", bufs=2) as cp,
                tc.tile_pool(name="rps", bufs=2, space="PSUM") as rps,
                tc.tile_pool(name="tps", bufs=2, space="PSUM") as tps,
            ):
                def emit_uchunk(btc):
                    # C rows for frames 4*btc..4*btc+3, PSUM at scale 2^22:
                    # fp16 main pass + two fp8-DoubleRow correction passes.
                    src_h = xTh[:, btc * 128:(btc + 1) * 128]
                    src_8h = xT8h[:, btc * 128:(btc + 1) * 128]
                    src_8l = xT8l[:, btc * 128:(btc + 1) * 128]
                    xch = ustg.tile([128, KIN], FP16, tag="xch", name="xch")
                    nc.sync.dma_start(xch[:], src_h.rearrange("(kc p) j -> p kc j", p=128))
                    xc8h = ustg.tile([128, KC, 128], F8E4, tag="xc8h", name="xc8h")
                    nc.sync.dma_start(xc8h[:], src_8h.rearrange("(kc p) j -> p kc j", p=128))
                    xc8l = ustg.tile([128, KC, 128], F8E4, tag="xc8l", name="xc8l")
                    nc.sync.dma_start(xc8l[:], src_8l.rearrange("(kc p) j -> p kc j", p=128))
                    for hc in range(HC):
                        nsl = slice(hc * 512, hc * 512 + 512)
                        ut = ups.tile([128, 512], F32, tag="u", name="ut")
                        for kc in range(KC):
                            nc.tensor.matmul(
                                ut[:],
                                xch[:, kc * 128:(kc + 1) * 128],
                                Wh_sb[:, kc * N_HID + hc * 512:
                                      kc * N_HID + hc * 512 + 512],
                                start=(kc == 0),
                                stop=False,
                            )
                        for kp in range(KC // 2):
                            ks = slice(2 * kp, 2 * kp + 2)
                            nc.tensor.matmul(
                                ut[:], xc8l[:, ks, :], W8h_sb[:, ks, nsl],
                                start=False, stop=False, perf_mode=DR,
                            )
                        for kp in range(KC // 2):
                            ks = slice(2 * kp, 2 * kp + 2)
                            nc.tensor.matmul(
                                ut[:], xc8h[:, ks, :], W8l_sb[:, ks, nsl],
                                start=False, stop=(kp == KC // 2 - 1), perf_mode=DR,
                            )
                        # unscale on the ACT engine: keeps the DVE FIFO free of
                        # cst drains (a DVE cst behind frame-tail ops stalled
                        # the U matmuls on PSUM-buf reuse -> HAM re-throttle).
                        # bufs=2: a bufs=1 cst serialized ACT on the C-write
                        # DMA (WAR), head-blocking the ACT FIFO and stalling
                        # U matmuls on PSUM reuse.
                        cst = ustg.tile([128, 512], F32, tag="cst", name="cst", bufs=2)
                        nc.scalar.activation(
                            cst[:], ut[:], mybir.ActivationFunctionType.Copy,
                            scale=USC,
                        )
                        nc.sync.dma_start(
                            C_d[4 * btc:4 * btc + 4, hc * 32:hc * 32 + 32, :],
                            cst[:],
                        )

                mem_t = [st.tile([128, 512], F32, tag=f"mem{i}", name=f"mem{i}") for i in range(2)]
                spk_t = [st.tile([128, 512], BF16, tag=f"spk{i}", name=f"spk{i}") for i in range(2)]
                h_sb = st.tile([128, 512], F32, tag="h")
                sph_sb = st.tile([128, 512], F32, tag="sph")
                tmh_sb = st.tile([128, 512], F32, tag="tmh")

                # h for frame 0: spike=0 -> h = 0.5*mem0 - 0.5
                nc.vector.tensor_scalar(
                    h_sb[:], mem0_sb[:], 0.5, -0.5,
                    mybir.AluOpType.mult, mybir.AluOpType.add,
                )
                nc.vector.tensor_scalar(
                    tmh_sb[:], h_sb[:], -1.0, 0.5,
                    mybir.AluOpType.mult, mybir.AluOpType.add,
                )

                # A right after W on the scalar queue and BEFORE the PRE
                # chunks' csts (an ACT cst waiting on matmuls would block the
                # A DMA issues for tens of us).  All of A_hi before any A_lo,
                # in the recurrence's kt consumption order (kt = 4*kk + q,
                # q outer), so frame 1's hi pass streams as tiles land.
                kt_order = [4 * kk + q for q in range(4) for kk in range(4)]
                for kt in kt_order:
                    sl = slice(kt * N_HID, (kt + 1) * N_HID)
                    nc.scalar.dma_start(A_hi[:, sl], Aph[:, sl])
                for kt in kt_order:
                    sl = slice(kt * N_HID, (kt + 1) * N_HID)
                    nc.scalar.dma_start(A_lo[:, sl], Apl[:, sl])

                PRE = 4
                for btc in range(min(PRE, btc_used)):
                    emit_uchunk(btc)

                for t in range(frames):
                    cur, nxt = t % 2, (t + 1) % 2
                    c_q = cp.tile([128, 512], F32, tag="c")
                    nc.sync.dma_start(c_q[:], C_d[t])

                    if t == 0:
                        t1f = c_q[:]
                    else:
                        # col-tiled quads: the 4 output chunks run concurrently
                        # in the 4 column-groups of the PE array (M=32 each).
                        # kt ordered by (kt%4) so the stationary consumes the
                        # previous frame's spike blocks in production order.
                        r_ps = rps.tile([128, 512], F32, tag="r")
                        for pi, A_h in enumerate((A_hi, A_lo)):
                            for q in range(4):
                                for kk in range(4):
                                    kt = 4 * kk + q
                                    so = q * 128 + kk * 32
                                    first = pi == 0 and q == 0 and kk == 0
                                    last = pi == 1 and q == 3 and kk == 3
                                    for hc in range(HC):
                                        nc.tensor.matmul(
                                            r_ps[hc * 32:(hc + 1) * 32, :],
                                            spk_t[cur][:, so:so + 32],
                                            A_h[:, kt * N_HID + hc * 512: kt * N_HID + hc * 512 + 512],
                                            start=first,
                                            stop=last,
                                            tile_position=(0, hc * 32),
                                            skip_group_check=True,
                                        )
                        t1t = fs.tile([128, 512], F32, tag="t1", bufs=1)
                        t1f = t1t[:]

                    y_hm = fs.tile([128, 512], F32, tag="yhm", bufs=1)
                    # per-block pipeline: add-C, transpose, tanh (from PSUM),
                    # threshold -- each 128-wide block flows independently so
                    # the next frame's matmuls can start on early blocks.
                    for j in range(4):
                        blk = slice(j * 128, (j + 1) * 128)
                        if t > 0:
                            nc.vector.tensor_add(t1f[:, blk], r_ps[:, blk], c_q[:, blk])
                        tp = tps.tile([128, 128], F32, tag="tp", name="tp")
                        nc.tensor.transpose(tp[:], t1f[:, blk], eye_sb[:])
                        nc.scalar.activation(
                            y_hm[:, blk], tp[:], mybir.ActivationFunctionType.Tanh
                        )
                        nc.vector.tensor_tensor(
                            spk_t[nxt][:, blk], y_hm[:, blk], tmh_sb[:, blk],
                            op=mybir.AluOpType.is_gt,
                        )

                    # off the spike chain: mem' = y + h, then next h and 0.5-h
                    # (h updated in place once the old value is consumed)
                    nc.vector.tensor_add(mem_t[nxt][:], y_hm[:], h_sb[:])
                    if t + 1 < frames:
                        nc.vector.tensor_scalar(
                            sph_sb[:], mem_t[nxt][:], THR, 0.5,
                            mybir.AluOpType.is_gt, mybir.AluOpType.mult,
                        )
                        nc.vector.tensor_scalar(
                            h_sb[:], mem_t[nxt][:], 0.5, -0.5,
                            mybir.AluOpType.mult, mybir.AluOpType.add,
                        )
                        nc.vector.tensor_add(h_sb[:], h_sb[:], sph_sb[:])
                        nc.vector.tensor_scalar(
                            tmh_sb[:], h_sb[:], -1.0, 0.5,
                            mybir.AluOpType.mult, mybir.AluOpType.add,
                        )

                    nc.sync.dma_start(memsT[t], mem_t[nxt][:])
                    nc.sync.dma_start(spikesT[t], spk_t[nxt][:])

                    # U chunk AFTER the frame's ops: its matmuls fill the PE
                    # during the frame tail, and its csts queue behind (not
                    # ahead of) this frame's tanhs on the ACT engine.
                    if t % 4 == 0 and t // 4 + PRE < btc_used:
                        emit_uchunk(t // 4 + PRE)

    nc.compile()
    return nc


def _host_prep(x, W_in, A, bias, mem_init, frames):
    """Build per-core input maps (shared arrays computed once)."""
    x = np.ascontiguousarray(x, dtype=np.float32)
    W_in = np.asarray(W_in, dtype=np.float32)
    A = np.asarray(A, dtype=np.float32)
    bias = np.asarray(bias, dtype=np.float32)
    mem_init = np.asarray(mem_init, dtype=np.float32)

    import ml_dtypes

    Apf = (ALPHA * A).reshape(KT, 128, N_HID).transpose(1, 0, 2).reshape(128, KT * N_HID)
    Apf = np.ascontiguousarray(Apf)
    Aph = Apf.astype(ml_dtypes.bfloat16)
    Apl = (Apf - Aph.astype(np.float32)).astype(ml_dtypes.bfloat16)

    # W at scale 2^11; fp16 hi + residual for the fp8 passes.
    W_aug = np.zeros((KIN, N_HID), dtype=np.float32)
    W_aug[:N_IN] = (1.0 - ALPHA) * W_in
    W_aug[N_IN] = (1.0 - ALPHA) * bias
    # mybir float8e4 == ml_dtypes.float8_e4m3 (IEEE-style, max 240, has inf):
    # clip before cast so a tail value can never encode as inf.
    def e4m3(a):
        return np.clip(a, -224.0, 224.0).astype(ml_dtypes.float8_e4m3)

    Ws = W_aug * np.float32(2.0 ** 11)
    Wh = Ws.astype(np.float16)                       # p1 operand
    Wl = Ws - Wh.astype(np.float32)
    W8h = e4m3(Ws * np.float32(2.0 ** -3))           # W*2^8  (max ~128)
    W8l = e4m3(Wl * np.float32(2.0 ** 6))            # Wl*2^17 (max ~32)

    def pack_w(Wm):
        return np.ascontiguousarray(
            Wm.reshape(KC, 128, N_HID).transpose(1, 0, 2))

    Wph = np.ascontiguousarray(
        Wh.reshape(KC, 128, N_HID).transpose(1, 0, 2).reshape(128, KC * N_HID))
    Wp8h = pack_w(W8h)
    Wp8l = pack_w(W8l)

    eye = np.eye(128, dtype=np.float32)

    in_maps = []
    for i in range(NCORES):
        xs = x[i * B:(i + 1) * B, :frames]            # [B, frames, N_IN]
        xTc = np.zeros((KIN, B * frames), dtype=np.float32)
        # xT[n, t*B + b] = x[b, t, n]
        xTc[:N_IN] = xs.transpose(2, 1, 0).reshape(N_IN, frames * B)
        xTc[N_IN] = 1.0
        if frames < FRAMES:
            full = np.zeros((KIN, BT), dtype=np.float32)
            full[:, : B * frames] = xTc
            xTc = full
        def e4m3(a):
            return np.clip(a, -224.0, 224.0).astype(ml_dtypes.float8_e4m3)

        xss = xTc * np.float32(2.0 ** 11)             # x*2^11
        xTh16 = xss.astype(np.float16)
        xl = xss - xTh16.astype(np.float32)
        xT8h = e4m3(xTh16.astype(np.float32) * np.float32(2.0 ** -6))  # fp16(x)*2^5
        xT8l = e4m3(xl.astype(np.float16).astype(np.float32)
                    * np.float32(2.0 ** 3))           # fp16(xl)*2^14
        ms = mem_init[i * B:(i + 1) * B]              # [B, N_HID]
        # hm packing: hm[p, q*128 + hc*32 + b] = mem[b, hc*512 + q*128 + p]
        m0 = ms.reshape(B, 4, 4, 128).transpose(3, 2, 1, 0).reshape(128, 512)
        in_maps.append(
            {
                "Aph": Aph,
                "Apl": Apl,
                "Wph": Wph,
                "Wp8h": Wp8h,
                "Wp8l": Wp8l,
                "xTh": np.ascontiguousarray(xTh16),
                "xT8h": np.ascontiguousarray(xT8h),
                "xT8l": np.ascontiguousarray(xT8l),
                "mem0": np.ascontiguousarray(m0),
                "eye": eye,
            }
        )
    return in_maps


def kernel(x, W_in, A, bias, mem_init):
    global LAST_RESULT
    frames = int(os.environ.get("ANNRC_FRAMES", FRAMES))

    if frames not in _NC_CACHE:
        _NC_CACHE[frames] = _build_nc(frames)
    nc = _NC_CACHE[frames]

    in_maps = _host_prep(x, W_in, A, bias, mem_init, frames)
    res = run_bass_kernel_spmd(nc, in_maps, core_ids=list(range(NCORES)))
    LAST_RESULT = res

    mems = np.empty((BATCH, frames, N_HID), dtype=np.float32)
    spikes = np.empty((BATCH, frames, N_HID), dtype=np.float32)
    for i in range(NCORES):
        out = res.results[i]
        mt = out["memsT"].reshape(frames, 128, 4, 4, B).transpose(4, 0, 3, 2, 1)
        mems[i * B:(i + 1) * B] = mt.reshape(B, frames, N_HID)
        sp = np.asarray(out["spikesT"], np.float32).reshape(frames, 128, 4, 4, B)
        spikes[i * B:(i + 1) * B] = sp.transpose(4, 0, 3, 2, 1).reshape(B, frames, N_HID)
    return mems, spikes


# revision 23
# speedup vs baseline: 1.0291x; 1.0291x over previous
"""Trainium2 Bass kernel for the AnnRC spiking-RNN problem.

Strategy: pure data parallelism across batch (8 cores x 32 rows each).

Input matmul (C = 0.5*(x @ W_in + bias), all frames), computed at PSUM
scale 2^22 so three passes share one accumulation group:
  p1: fp16(x*2^11) @ fp16(W*2^11)            (exact products)
  p2: e4m3(xl*2^13) @ e4m3(W*2^9)   fp8 DoubleRow, 0.5 cyc/row
  p3: e4m3(x*2^5)   @ e4m3(Wl*2^17) fp8 DoubleRow, 0.5 cyc/row
(xl = x - fp16(x), Wl = W - fp16(W).)  The 2^-22 unscale rides the
PSUM->SBUF copy (tensor_scalar mult).  numpy sim (sim_precision.py):
319 spike flips vs the ~476-flip budget (rel-err gate 2e-2); the sim
reproduces hw exactly (baseline scheme 43 sim vs 40 hw, fp16-A 1963 vs
1964).  DoubleRow pairs two k-tiles per instruction ([128,2,f] APs, both
operands fp8e4) and is only legal WITHOUT tile_position col-tiling,
which is why it applies to the U passes (M=128) and not the recurrence.

Recurrence per frame:
  r' = spike @ (0.5*A)  -- A as an exact bf16 hi+lo split, two
       accumulating passes (single-pass fp16/bf16 A fails: ~1964 flips).
       The four 512-wide output chunks run CONCURRENTLY in the four
       column groups of the PE array via tile_position (M=32 each) --
       measured 216ns per 4x512-row quartet = full streaming roofline.
  Per 128-wide block j (pipelined):  t = r' + C_t (DVE add from PSUM);
  full 128x128 PE transpose into packed hidden-major order; y = tanh
  (ACT, reading PSUM);  spike' = (y > 0.5 - h)  (DVE is_gt, bf16 out ->
  feeds the next frame's stationary directly).
  Off the critical chain: mem' = y + h, h' = 0.5*mem' + 0.5*spike' - 0.5.

DMA: bulk weights (Wh/Wh8/Wl8, A hi/lo) issue on the SCALAR engine's
queue; streaming traffic (x chunks, C roundtrip, outputs) stays on SYNC
so frame 0's C read is not stuck behind 33MB of weight loads (the v1
prologue idled the PE for ~70us this way).  A DMAs are emitted after
the two PRE U-chunks so the first matmuls start as soon as W lands.

State is hidden-major packed [128, (j, hc, b)] so elementwise ops use all
128 partitions and spike slices feed the matmul stationary without any
per-frame transposition. Outputs are written packed and unpacked on host.
"""

import os
import sys
import types

import numpy as np

# ---------------------------------------------------------------------------
# antenv.axon_hooks shim: this image's antenv lacks the module, and
# concourse.bass_utils imports it unconditionally when tracing is requested.
if "antenv.axon_hooks" not in sys.modules:
    _hooks_mod = types.ModuleType("antenv.axon_hooks")
    _hooks_mod._hook = None
    _hooks_mod.set_axon_ntff_profile_hook = lambda h: setattr(_hooks_mod, "_hook", h)
    _hooks_mod.get_axon_ntff_profile_hook = lambda: _hooks_mod._hook
    sys.modules["antenv.axon_hooks"] = _hooks_mod
    try:
        from trn_agent_boot.trn_boot import _ntff_profile_via_ctypes

        _hooks_mod._hook = _ntff_profile_via_ctypes("/opt/axon/libaxon_pjrt.so")
    except Exception:
        pass

import concourse.bacc as bacc
import concourse.bass_utils as bass_utils
import concourse.mybir as mybir
import concourse.tile as tile
from concourse.bass_utils import run_bass_kernel_spmd

# Zero-egress container: artifact upload would fail; keep local.
bass_utils.upload_artifacts = lambda tmpdir: tmpdir

ALPHA, DECAY, THR = 0.5, 0.5, 0.5
N_IN, N_HID = 700, 2048
BATCH, FRAMES = 256, 128
NCORES = 8
B = BATCH // NCORES          # 32 batch rows per core
KT = N_HID // 128            # 16 k-tiles of the recurrent contraction
HC = N_HID // 512            # 4 n-chunks of 512
KIN = 768                    # padded input contraction (700 + 1 bias + pad)
KC = KIN // 128              # 6 k-tiles for the input matmul
BT = B * FRAMES              # 4096 (batch,frame) pairs per core
BTC = BT // 128              # 32 chunks of 128 bt-pairs

F32 = mybir.dt.float32
BF16 = mybir.dt.bfloat16
FP16 = mybir.dt.float16
F8E4 = mybir.dt.float8e4
DR = mybir.MatmulPerfMode.DoubleRow

USC = float(2.0 ** -22)      # PSUM unscale for the input matmul

LAST_RESULT = None  # test.py reads .exec_time_ns off this after a traced call

_NC_CACHE = {}


def _build_nc(frames):
    nc = bacc.Bacc("TRN2", target_bir_lowering=False, debug=False, num_devices=NCORES)

    Aph = nc.declare_dram_parameter("Aph", [128, KT * N_HID], BF16, isOutput=False)
    Apl = nc.declare_dram_parameter("Apl", [128, KT * N_HID], BF16, isOutput=False)
    Wph = nc.declare_dram_parameter("Wph", [128, KC * N_HID], FP16, isOutput=False)
    Wp8h = nc.declare_dram_parameter("Wp8h", [128, KC, N_HID], F8E4, isOutput=False)
    Wp8l = nc.declare_dram_parameter("Wp8l", [128, KC, N_HID], F8E4, isOutput=False)
    # x chunks stored per-btc contiguous: one 1536B/768B descriptor per
    # partition row instead of 768 x 256B strided ones (the strided form
    # cost ~3us of DMA-queue time per chunk and starved the c_q reads).
    xTh = nc.declare_dram_parameter("xTh", [BTC, 128, KIN], FP16, isOutput=False)
    xT8h = nc.declare_dram_parameter("xT8h", [BTC, 128, KC, 128], F8E4, isOutput=False)
    xT8l = nc.declare_dram_parameter("xT8l", [BTC, 128, KC, 128], F8E4, isOutput=False)
    mem0 = nc.declare_dram_parameter("mem0", [128, 512], F32, isOutput=False)
    eye = nc.declare_dram_parameter("eye", [128, 128], F32, isOutput=False)
    memsT = nc.declare_dram_parameter("memsT", [frames, 128, 512], F32, isOutput=True)
    spikesT = nc.declare_dram_parameter("spikesT", [frames, 128, 512], BF16, isOutput=True)

    btc_used = (B * frames + 127) // 128

    with tile.TileContext(nc) as tc:
        with (
            tc.tile_pool(name="dram", bufs=1, space="DRAM") as dram,
            tc.tile_pool(name="state", bufs=1) as st,
            tc.tile_pool(name="big", bufs=1) as big,
        ):
            C_d = dram.tile([FRAMES, 128, 512], F32, tag="C")
            eye_sb = st.tile([128, 128], F32, tag="eye")
            nc.scalar.dma_start(eye_sb[:], eye[:])
            mem0_sb = st.tile([128, 512], F32, tag="mem0")
            nc.scalar.dma_start(mem0_sb[:], mem0[:])

            # W first (per k-tile so the first matmuls start after ~2MB);
            # A is emitted AFTER the PRE U-chunks below, and all bulk
            # weights ride the scalar engine's DMA queue so the sync
            # queue (x chunks, C roundtrip, outputs) is never blocked.
            Wh_sb = big.tile([128, KC * N_HID], FP16, tag="Wh")
            W8h_sb = big.tile([128, KC, N_HID], F8E4, tag="W8h")
            W8l_sb = big.tile([128, KC, N_HID], F8E4, tag="W8l")
            for kc in range(KC):
                sl = slice(kc * N_HID, (kc + 1) * N_HID)
                nc.scalar.dma_start(Wh_sb[:, sl], Wph[:, sl])
                nc.scalar.dma_start(W8h_sb[:, kc, :], Wp8h[:, kc, :])
                nc.scalar.dma_start(W8l_sb[:, kc, :], Wp8l[:, kc, :])
            A_hi = big.tile([128, KT * N_HID], BF16, tag="Ahi")
            A_lo = big.tile([128, KT * N_HID], BF16, tag="Alo")

            with (
                tc.tile_pool(name="ustg", bufs=2) as ustg,
                tc.tile_pool(name="ups", bufs=4, space="PSUM") as ups,
                tc.tile_pool(name="fstg", bufs=2) as fs,
                tc.tile_pool(name="cpool", bufs=2) as cp,
                tc.tile_pool(name="rps", bufs=2, space="PSUM") as rps,
                tc.tile_pool(name="tps", bufs=2, space="PSUM") as tps,
            ):
                def emit_uchunk(btc):
                    # C rows for frames 4*btc..4*btc+3, PSUM at scale 2^22:
                    # fp16 main pass + two fp8-DoubleRow correction passes.
                    xch = ustg.tile([128, KIN], FP16, tag="xch", name="xch")
                    nc.sync.dma_start(xch[:], xTh[btc])
                    xc8h = ustg.tile([128, KC, 128], F8E4, tag="xc8h", name="xc8h")
                    nc.sync.dma_start(xc8h[:], xT8h[btc])
                    xc8l = ustg.tile([128, KC, 128], F8E4, tag="xc8l", name="xc8l")
                    nc.sync.dma_start(xc8l[:], xT8l[btc])
                    for hc in range(HC):
                        nsl = slice(hc * 512, hc * 512 + 512)
                        ut = ups.tile([128, 512], F32, tag="u", name="ut")
                        for kc in range(KC):
                            nc.tensor.matmul(
                                ut[:],
                                xch[:, kc * 128:(kc + 1) * 128],
                                Wh_sb[:, kc * N_HID + hc * 512:
                                      kc * N_HID + hc * 512 + 512],
                                start=(kc == 0),
                                stop=False,
                            )
                        for kp in range(KC // 2):
                            ks = slice(2 * kp, 2 * kp + 2)
                            nc.tensor.matmul(
                                ut[:], xc8l[:, ks, :], W8h_sb[:, ks, nsl],
                                start=False, stop=False, perf_mode=DR,
                            )
                        for kp in range(KC // 2):
                            ks = slice(2 * kp, 2 * kp + 2)
                            nc.tensor.matmul(
                                ut[:], xc8h[:, ks, :], W8l_sb[:, ks, nsl],
                                start=False, stop=(kp == KC // 2 - 1), perf_mode=DR,
                            )
                        # unscale on the ACT engine: keeps the DVE FIFO free of
                        # cst drains (a DVE cst behind frame-tail ops stalled
                        # the U matmuls on PSUM-buf reuse -> HAM re-throttle).
                        # bufs=2: a bufs=1 cst serialized ACT on the C-write
                        # DMA (WAR), head-blocking the ACT FIFO and stalling
                        # U matmuls on PSUM reuse.
                        cst = ustg.tile([128, 512], F32, tag="cst", name="cst", bufs=2)
                        nc.scalar.activation(
                            cst[:], ut[:], mybir.ActivationFunctionType.Copy,
                            scale=USC,
                        )
                        nc.sync.dma_start(
                            C_d[4 * btc:4 * btc + 4, hc * 32:hc * 32 + 32, :],
                            cst[:],
                        )

                mem_t = [st.tile([128, 512], F32, tag=f"mem{i}", name=f"mem{i}") for i in range(2)]
                spk_t = [st.tile([128, 512], BF16, tag=f"spk{i}", name=f"spk{i}") for i in range(2)]
                h_sb = st.tile([128, 512], F32, tag="h")
                sph_sb = st.tile([128, 512], F32, tag="sph")
                tmh_sb = st.tile([128, 512], F32, tag="tmh")

                # h for frame 0: spike=0 -> h = 0.5*mem0 - 0.5
                nc.vector.tensor_scalar(
                    h_sb[:], mem0_sb[:], 0.5, -0.5,
                    mybir.AluOpType.mult, mybir.AluOpType.add,
                )
                nc.vector.tensor_scalar(
                    tmh_sb[:], h_sb[:], -1.0, 0.5,
                    mybir.AluOpType.mult, mybir.AluOpType.add,
                )

                # A right after W on the scalar queue and BEFORE the PRE
                # chunks' csts (an ACT cst waiting on matmuls would block the
                # A DMA issues for tens of us).  All of A_hi before any A_lo,
                # in the recurrence's kt consumption order (kt = 4*kk + q,
                # q outer), so frame 1's hi pass streams as tiles land.
                kt_order = [4 * kk + q for q in range(4) for kk in range(4)]
                for kt in kt_order:
                    sl = slice(kt * N_HID, (kt + 1) * N_HID)
                    nc.scalar.dma_start(A_hi[:, sl], Aph[:, sl])
                for kt in kt_order:
                    sl = slice(kt * N_HID, (kt + 1) * N_HID)
                    nc.scalar.dma_start(A_lo[:, sl], Apl[:, sl])

                PRE = 4
                for btc in range(min(PRE, btc_used)):
                    emit_uchunk(btc)

                for t in range(frames):
                    cur, nxt = t % 2, (t + 1) % 2
                    c_q = cp.tile([128, 512], F32, tag="c")
                    nc.sync.dma_start(c_q[:], C_d[t])

                    if t == 0:
                        t1f = c_q[:]
                    else:
                        # col-tiled quads: the 4 output chunks run concurrently
                        # in the 4 column-groups of the PE array (M=32 each).
                        # kt ordered by (kt%4) so the stationary consumes the
                        # previous frame's spike blocks in production order.
                        r_ps = rps.tile([128, 512], F32, tag="r")
                        for pi, A_h in enumerate((A_hi, A_lo)):
                            for q in range(4):
                                for kk in range(4):
                                    kt = 4 * kk + q
                                    so = q * 128 + kk * 32
                                    first = pi == 0 and q == 0 and kk == 0
                                    last = pi == 1 and q == 3 and kk == 3
                                    for hc in range(HC):
                                        nc.tensor.matmul(
                                            r_ps[hc * 32:(hc + 1) * 32, :],
                                            spk_t[cur][:, so:so + 32],
                                            A_h[:, kt * N_HID + hc * 512: kt * N_HID + hc * 512 + 512],
                                            start=first,
                                            stop=last,
                                            tile_position=(0, hc * 32),
                                            skip_group_check=True,
                                        )
                        t1t = fs.tile([128, 512], F32, tag="t1", bufs=1)
                        t1f = t1t[:]

                    y_hm = fs.tile([128, 512], F32, tag="yhm", bufs=1)
                    # per-block pipeline: add-C, transpose, tanh (from PSUM),
                    # threshold -- each 128-wide block flows independently so
                    # the next frame's matmuls can start on early blocks.
                    for j in range(4):
                        blk = slice(j * 128, (j + 1) * 128)
                        if t > 0:
                            nc.vector.tensor_add(t1f[:, blk], r_ps[:, blk], c_q[:, blk])
                        tp = tps.tile([128, 128], F32, tag="tp", name="tp")
                        nc.tensor.transpose(tp[:], t1f[:, blk], eye_sb[:])
                        nc.scalar.activation(
                            y_hm[:, blk], tp[:], mybir.ActivationFunctionType.Tanh
                        )
                        nc.vector.tensor_tensor(
                            spk_t[nxt][:, blk], y_hm[:, blk], tmh_sb[:, blk],
                            op=mybir.AluOpType.is_gt,
                        )

                    # off the spike chain: mem' = y + h, then next h and 0.5-h
                    # (h updated in place once the old value is consumed)
                    nc.vector.tensor_add(mem_t[nxt][:], y_hm[:], h_sb[:])
                    if t + 1 < frames:
                        nc.vector.tensor_scalar(
                            sph_sb[:], mem_t[nxt][:], THR, 0.5,
                            mybir.AluOpType.is_gt, mybir.AluOpType.mult,
                        )
                        nc.vector.tensor_scalar(
                            h_sb[:], mem_t[nxt][:], 0.5, -0.5,
                            mybir.AluOpType.mult, mybir.AluOpType.add,
                        )
                        nc.vector.tensor_add(h_sb[:], h_sb[:], sph_sb[:])
                        nc.vector.tensor_scalar(
                            tmh_sb[:], h_sb[:], -1.0, 0.5,
                            mybir.AluOpType.mult, mybir.AluOpType.add,
                        )

                    # outputs ride the scalar queue so they never delay the
                    # c_q / x reads on the sync queue
                    nc.scalar.dma_start(memsT[t], mem_t[nxt][:])
                    nc.scalar.dma_start(spikesT[t], spk_t[nxt][:])

                    # U chunk AFTER the frame's ops: its matmuls fill the PE
                    # during the frame tail, and its csts queue behind (not
                    # ahead of) this frame's tanhs on the ACT engine.
                    if t % 4 == 0 and t // 4 + PRE < btc_used:
                        emit_uchunk(t // 4 + PRE)

    nc.compile()
    return nc


def _host_prep(x, W_in, A, bias, mem_init, frames):
    """Build per-core input maps (shared arrays computed once)."""
    x = np.ascontiguousarray(x, dtype=np.float32)
    W_in = np.asarray(W_in, dtype=np.float32)
    A = np.asarray(A, dtype=np.float32)
    bias = np.asarray(bias, dtype=np.float32)
    mem_init = np.asarray(mem_init, dtype=np.float32)

    import ml_dtypes

    Apf = (ALPHA * A).reshape(KT, 128, N_HID).transpose(1, 0, 2).reshape(128, KT * N_HID)
    Apf = np.ascontiguousarray(Apf)
    Aph = Apf.astype(ml_dtypes.bfloat16)
    Apl = (Apf - Aph.astype(np.float32)).astype(ml_dtypes.bfloat16)

    # W at scale 2^11; fp16 hi + residual for the fp8 passes.
    W_aug = np.zeros((KIN, N_HID), dtype=np.float32)
    W_aug[:N_IN] = (1.0 - ALPHA) * W_in
    W_aug[N_IN] = (1.0 - ALPHA) * bias
    # mybir float8e4 == ml_dtypes.float8_e4m3 (IEEE-style, max 240, has inf):
    # clip before cast so a tail value can never encode as inf.
    def e4m3(a):
        return np.clip(a, -224.0, 224.0).astype(ml_dtypes.float8_e4m3)

    Ws = W_aug * np.float32(2.0 ** 11)
    Wh = Ws.astype(np.float16)                       # p1 operand
    Wl = Ws - Wh.astype(np.float32)
    W8h = e4m3(Ws * np.float32(2.0 ** -3))           # W*2^8  (max ~128)
    W8l = e4m3(Wl * np.float32(2.0 ** 6))            # Wl*2^17 (max ~32)

    def pack_w(Wm):
        return np.ascontiguousarray(
            Wm.reshape(KC, 128, N_HID).transpose(1, 0, 2))

    Wph = np.ascontiguousarray(
        Wh.reshape(KC, 128, N_HID).transpose(1, 0, 2).reshape(128, KC * N_HID))
    Wp8h = pack_w(W8h)
    Wp8l = pack_w(W8l)

    eye = np.eye(128, dtype=np.float32)

    in_maps = []
    for i in range(NCORES):
        xs = x[i * B:(i + 1) * B, :frames]            # [B, frames, N_IN]
        xTc = np.zeros((KIN, B * frames), dtype=np.float32)
        # xT[n, t*B + b] = x[b, t, n]
        xTc[:N_IN] = xs.transpose(2, 1, 0).reshape(N_IN, frames * B)
        xTc[N_IN] = 1.0
        if frames < FRAMES:
            full = np.zeros((KIN, BT), dtype=np.float32)
            full[:, : B * frames] = xTc
            xTc = full
        def e4m3(a):
            return np.clip(a, -224.0, 224.0).astype(ml_dtypes.float8_e4m3)

        def pack_x(a):
            # [KIN, BT] -> [BTC, 128(p), KC, 128(j)] contiguous per chunk
            return np.ascontiguousarray(
                a.reshape(KC, 128, BTC, 128).transpose(2, 1, 0, 3))

        xss = xTc * np.float32(2.0 ** 11)             # x*2^11
        xTh16 = xss.astype(np.float16)
        xl = xss - xTh16.astype(np.float32)
        xT8h = e4m3(xTh16.astype(np.float32) * np.float32(2.0 ** -6))  # fp16(x)*2^5
        xT8l = e4m3(xl.astype(np.float16).astype(np.float32)
                    * np.float32(2.0 ** 3))           # fp16(xl)*2^14
        xTh16 = pack_x(xTh16).reshape(BTC, 128, KIN)
        xT8h = pack_x(xT8h)
        xT8l = pack_x(xT8l)
        ms = mem_init[i * B:(i + 1) * B]              # [B, N_HID]
        # hm packing: hm[p, q*128 + hc*32 + b] = mem[b, hc*512 + q*128 + p]
        m0 = ms.reshape(B, 4, 4, 128).transpose(3, 2, 1, 0).reshape(128, 512)
        in_maps.append(
            {
                "Aph": Aph,
                "Apl": Apl,
                "Wph": Wph,
                "Wp8h": Wp8h,
                "Wp8l": Wp8l,
                "xTh": np.ascontiguousarray(xTh16),
                "xT8h": np.ascontiguousarray(xT8h),
                "xT8l": np.ascontiguousarray(xT8l),
                "mem0": np.ascontiguousarray(m0),
                "eye": eye,
            }
        )
    return in_maps


def kernel(x, W_in, A, bias, mem_init):
    global LAST_RESULT
    frames = int(os.environ.get("ANNRC_FRAMES", FRAMES))

    if frames not in _NC_CACHE:
        _NC_CACHE[frames] = _build_nc(frames)
    nc = _NC_CACHE[frames]

    in_maps = _host_prep(x, W_in, A, bias, mem_init, frames)
    res = run_bass_kernel_spmd(nc, in_maps, core_ids=list(range(NCORES)))
    LAST_RESULT = res

    mems = np.empty((BATCH, frames, N_HID), dtype=np.float32)
    spikes = np.empty((BATCH, frames, N_HID), dtype=np.float32)
    for i in range(NCORES):
        out = res.results[i]
        mt = out["memsT"].reshape(frames, 128, 4, 4, B).transpose(4, 0, 3, 2, 1)
        mems[i * B:(i + 1) * B] = mt.reshape(B, frames, N_HID)
        sp = np.asarray(out["spikesT"], np.float32).reshape(frames, 128, 4, 4, B)
        spikes[i * B:(i + 1) * B] = sp.transpose(4, 0, 3, 2, 1).reshape(B, frames, N_HID)
    return mems, spikes


# revision 26
# speedup vs baseline: 1.0479x; 1.0182x over previous
"""Trainium2 Bass kernel for the AnnRC spiking-RNN problem.

Strategy: pure data parallelism across batch (8 cores x 32 rows each).

Input matmul (C = 0.5*(x @ W_in + bias), all frames), computed at PSUM
scale 2^22 so three passes share one accumulation group:
  p1: fp16(x*2^11) @ fp16(W*2^11)            (exact products)
  p2: e4m3(xl*2^13) @ e4m3(W*2^9)   fp8 DoubleRow, 0.5 cyc/row
  p3: e4m3(x*2^5)   @ e4m3(Wl*2^17) fp8 DoubleRow, 0.5 cyc/row
(xl = x - fp16(x), Wl = W - fp16(W).)  The 2^-22 unscale rides the
PSUM->SBUF copy (tensor_scalar mult).  numpy sim (sim_precision.py):
319 spike flips vs the ~476-flip budget (rel-err gate 2e-2); the sim
reproduces hw exactly (baseline scheme 43 sim vs 40 hw, fp16-A 1963 vs
1964).  DoubleRow pairs two k-tiles per instruction ([128,2,f] APs, both
operands fp8e4) and is only legal WITHOUT tile_position col-tiling,
which is why it applies to the U passes (M=128) and not the recurrence.

Recurrence per frame:
  r' = spike @ (0.5*A)  -- A as an exact bf16 hi+lo split, two
       accumulating passes (single-pass fp16/bf16 A fails: ~1964 flips).
       The four 512-wide output chunks run CONCURRENTLY in the four
       column groups of the PE array via tile_position (M=32 each) --
       measured 216ns per 4x512-row quartet = full streaming roofline.
  Per 128-wide block j (pipelined):  t = r' + C_t (DVE add from PSUM);
  full 128x128 PE transpose into packed hidden-major order; y = tanh
  (ACT, reading PSUM);  spike' = (y > 0.5 - h)  (DVE is_gt, bf16 out ->
  feeds the next frame's stationary directly).
  Off the critical chain: mem' = y + h, h' = 0.5*mem' + 0.5*spike' - 0.5.

DMA: bulk weights (Wh/Wh8/Wl8, A hi/lo) issue on the SCALAR engine's
queue; streaming traffic (x chunks, C roundtrip, outputs) stays on SYNC
so frame 0's C read is not stuck behind 33MB of weight loads (the v1
prologue idled the PE for ~70us this way).  A DMAs are emitted after
the two PRE U-chunks so the first matmuls start as soon as W lands.

State is hidden-major packed [128, (j, hc, b)] so elementwise ops use all
128 partitions and spike slices feed the matmul stationary without any
per-frame transposition. Outputs are written packed and unpacked on host.
"""

import os
import sys
import types

import numpy as np

# ---------------------------------------------------------------------------
# antenv.axon_hooks shim: this image's antenv lacks the module, and
# concourse.bass_utils imports it unconditionally when tracing is requested.
if "antenv.axon_hooks" not in sys.modules:
    _hooks_mod = types.ModuleType("antenv.axon_hooks")
    _hooks_mod._hook = None
    _hooks_mod.set_axon_ntff_profile_hook = lambda h: setattr(_hooks_mod, "_hook", h)
    _hooks_mod.get_axon_ntff_profile_hook = lambda: _hooks_mod._hook
    sys.modules["antenv.axon_hooks"] = _hooks_mod
    try:
        from trn_agent_boot.trn_boot import _ntff_profile_via_ctypes

        _hooks_mod._hook = _ntff_profile_via_ctypes("/opt/axon/libaxon_pjrt.so")
    except Exception:
        pass

import concourse.bacc as bacc
import concourse.bass_utils as bass_utils
import concourse.mybir as mybir
import concourse.tile as tile
from concourse.bass_utils import run_bass_kernel_spmd

# Zero-egress container: artifact upload would fail; keep local.
bass_utils.upload_artifacts = lambda tmpdir: tmpdir

ALPHA, DECAY, THR = 0.5, 0.5, 0.5
N_IN, N_HID = 700, 2048
BATCH, FRAMES = 256, 128
NCORES = 8
B = BATCH // NCORES          # 32 batch rows per core
KT = N_HID // 128            # 16 k-tiles of the recurrent contraction
HC = N_HID // 512            # 4 n-chunks of 512
KIN = 768                    # padded input contraction (700 + 1 bias + pad)
KC = KIN // 128              # 6 k-tiles for the input matmul
BT = B * FRAMES              # 4096 (batch,frame) pairs per core
BTC = BT // 128              # 32 chunks of 128 bt-pairs

F32 = mybir.dt.float32
BF16 = mybir.dt.bfloat16
FP16 = mybir.dt.float16
F8E4 = mybir.dt.float8e4
DR = mybir.MatmulPerfMode.DoubleRow

USC = float(2.0 ** -22)      # PSUM unscale for the input matmul

LAST_RESULT = None  # test.py reads .exec_time_ns off this after a traced call

_NC_CACHE = {}


def _build_nc(frames):
    nc = bacc.Bacc("TRN2", target_bir_lowering=False, debug=False, num_devices=NCORES)

    Aph = nc.declare_dram_parameter("Aph", [128, KT * N_HID], BF16, isOutput=False)
    Apl = nc.declare_dram_parameter("Apl", [128, KT * N_HID], BF16, isOutput=False)
    Wph = nc.declare_dram_parameter("Wph", [128, KC * N_HID], FP16, isOutput=False)
    Wp8h = nc.declare_dram_parameter("Wp8h", [128, KC, N_HID], F8E4, isOutput=False)
    Wp8l = nc.declare_dram_parameter("Wp8l", [128, KC, N_HID], F8E4, isOutput=False)
    # x chunks stored per-btc contiguous: one 1536B/768B descriptor per
    # partition row instead of 768 x 256B strided ones (the strided form
    # cost ~3us of DMA-queue time per chunk and starved the c_q reads).
    xTh = nc.declare_dram_parameter("xTh", [BTC, 128, KIN], FP16, isOutput=False)
    xT8h = nc.declare_dram_parameter("xT8h", [BTC, 128, KC, 128], F8E4, isOutput=False)
    xT8l = nc.declare_dram_parameter("xT8l", [BTC, 128, KC, 128], F8E4, isOutput=False)
    mem0 = nc.declare_dram_parameter("mem0", [128, 512], F32, isOutput=False)
    eye = nc.declare_dram_parameter("eye", [128, 128], F32, isOutput=False)
    memsT = nc.declare_dram_parameter("memsT", [frames, 128, 512], F32, isOutput=True)
    spikesT = nc.declare_dram_parameter("spikesT", [frames, 128, 512], BF16, isOutput=True)

    btc_used = (B * frames + 127) // 128

    with tile.TileContext(nc) as tc:
        with (
            tc.tile_pool(name="dram", bufs=1, space="DRAM") as dram,
            tc.tile_pool(name="state", bufs=1) as st,
            tc.tile_pool(name="big", bufs=1) as big,
        ):
            C_d = dram.tile([FRAMES, 128, 512], F32, tag="C")
            eye_sb = st.tile([128, 128], F32, tag="eye")
            nc.scalar.dma_start(eye_sb[:], eye[:])
            mem0_sb = st.tile([128, 512], F32, tag="mem0")
            nc.scalar.dma_start(mem0_sb[:], mem0[:])

            # W first (per k-tile so the first matmuls start after ~2MB);
            # A is emitted AFTER the PRE U-chunks below, and all bulk
            # weights ride the scalar engine's DMA queue so the sync
            # queue (x chunks, C roundtrip, outputs) is never blocked.
            Wh_sb = big.tile([128, KC * N_HID], FP16, tag="Wh")
            W8h_sb = big.tile([128, KC, N_HID], F8E4, tag="W8h")
            W8l_sb = big.tile([128, KC, N_HID], F8E4, tag="W8l")
            for kc in range(KC):
                sl = slice(kc * N_HID, (kc + 1) * N_HID)
                nc.scalar.dma_start(Wh_sb[:, sl], Wph[:, sl])
                nc.scalar.dma_start(W8h_sb[:, kc, :], Wp8h[:, kc, :])
                nc.scalar.dma_start(W8l_sb[:, kc, :], Wp8l[:, kc, :])
            A_hi = big.tile([128, KT * N_HID], BF16, tag="Ahi")
            A_lo = big.tile([128, KT * N_HID], BF16, tag="Alo")

            with (
                tc.tile_pool(name="ustg", bufs=2) as ustg,
                tc.tile_pool(name="ups", bufs=4, space="PSUM") as ups,
                tc.tile_pool(name="fstg", bufs=2) as fs,
                tc.tile_pool(name="cpool", bufs=2) as cp,
                tc.tile_pool(name="rps", bufs=2, space="PSUM") as rps,
                tc.tile_pool(name="tps", bufs=2, space="PSUM") as tps,
            ):
                def emit_uchunk(btc):
                    # C rows for frames 4*btc..4*btc+3, PSUM at scale 2^22:
                    # fp16 main pass + two fp8-DoubleRow correction passes.
                    # sync queue is reserved for the latency-critical c_q
                    # reads; x chunks ride the scalar queue (8-frame lead)
                    xch = ustg.tile([128, KIN], FP16, tag="xch", name="xch")
                    nc.scalar.dma_start(xch[:], xTh[btc])
                    xc8h = ustg.tile([128, KC, 128], F8E4, tag="xc8h", name="xc8h")
                    nc.scalar.dma_start(xc8h[:], xT8h[btc])
                    xc8l = ustg.tile([128, KC, 128], F8E4, tag="xc8l", name="xc8l")
                    nc.scalar.dma_start(xc8l[:], xT8l[btc])
                    for hc in range(HC):
                        nsl = slice(hc * 512, hc * 512 + 512)
                        ut = ups.tile([128, 512], F32, tag="u", name="ut")
                        for kc in range(KC):
                            nc.tensor.matmul(
                                ut[:],
                                xch[:, kc * 128:(kc + 1) * 128],
                                Wh_sb[:, kc * N_HID + hc * 512:
                                      kc * N_HID + hc * 512 + 512],
                                start=(kc == 0),
                                stop=False,
                            )
                        for kp in range(KC // 2):
                            ks = slice(2 * kp, 2 * kp + 2)
                            nc.tensor.matmul(
                                ut[:], xc8l[:, ks, :], W8h_sb[:, ks, nsl],
                                start=False, stop=False, perf_mode=DR,
                            )
                        for kp in range(KC // 2):
                            ks = slice(2 * kp, 2 * kp + 2)
                            nc.tensor.matmul(
                                ut[:], xc8h[:, ks, :], W8l_sb[:, ks, nsl],
                                start=False, stop=(kp == KC // 2 - 1), perf_mode=DR,
                            )
                        # unscale on the ACT engine: keeps the DVE FIFO free of
                        # cst drains (a DVE cst behind frame-tail ops stalled
                        # the U matmuls on PSUM-buf reuse -> HAM re-throttle).
                        # bufs=2: a bufs=1 cst serialized ACT on the C-write
                        # DMA (WAR), head-blocking the ACT FIFO and stalling
                        # U matmuls on PSUM reuse.
                        cst = ustg.tile([128, 512], F32, tag="cst", name="cst", bufs=2)
                        nc.scalar.activation(
                            cst[:], ut[:], mybir.ActivationFunctionType.Copy,
                            scale=USC,
                        )
                        # C write directly after its cst on the scalar queue
                        # (zero extra wait); keeps the sync queue to c_q only
                        # (the sync descriptor ring wrap-drained every ~24
                        # DMAs and starved c_q -> periodic PE gaps).
                        nc.scalar.dma_start(
                            C_d[4 * btc:4 * btc + 4, hc * 32:hc * 32 + 32, :],
                            cst[:],
                        )

                mem_t = [st.tile([128, 512], F32, tag=f"mem{i}", name=f"mem{i}") for i in range(2)]
                spk_t = [st.tile([128, 512], BF16, tag=f"spk{i}", name=f"spk{i}") for i in range(2)]
                h_sb = st.tile([128, 512], F32, tag="h")
                sph_sb = st.tile([128, 512], F32, tag="sph")
                tmh_sb = st.tile([128, 512], F32, tag="tmh")

                # h for frame 0: spike=0 -> h = 0.5*mem0 - 0.5
                nc.vector.tensor_scalar(
                    h_sb[:], mem0_sb[:], 0.5, -0.5,
                    mybir.AluOpType.mult, mybir.AluOpType.add,
                )
                nc.vector.tensor_scalar(
                    tmh_sb[:], h_sb[:], -1.0, 0.5,
                    mybir.AluOpType.mult, mybir.AluOpType.add,
                )

                # A right after W on the scalar queue and BEFORE the PRE
                # chunks' csts (an ACT cst waiting on matmuls would block the
                # A DMA issues for tens of us).  All of A_hi before any A_lo,
                # in the recurrence's kt consumption order (kt = 4*kk + q,
                # q outer), so frame 1's hi pass streams as tiles land.
                kt_order = [4 * kk + q for q in range(4) for kk in range(4)]
                for kt in kt_order:
                    sl = slice(kt * N_HID, (kt + 1) * N_HID)
                    nc.scalar.dma_start(A_hi[:, sl], Aph[:, sl])
                for kt in kt_order:
                    sl = slice(kt * N_HID, (kt + 1) * N_HID)
                    nc.scalar.dma_start(A_lo[:, sl], Apl[:, sl])

                PRE = 4
                for btc in range(min(PRE, btc_used)):
                    emit_uchunk(btc)

                for t in range(frames):
                    cur, nxt = t % 2, (t + 1) % 2
                    c_q = cp.tile([128, 512], F32, tag="c")
                    nc.sync.dma_start(c_q[:], C_d[t])

                    if t == 0:
                        t1f = c_q[:]
                    else:
                        # col-tiled quads: the 4 output chunks run concurrently
                        # in the 4 column-groups of the PE array (M=32 each).
                        # kt ordered by (kt%4) so the stationary consumes the
                        # previous frame's spike blocks in production order.
                        r_ps = rps.tile([128, 512], F32, tag="r")
                        for pi, A_h in enumerate((A_hi, A_lo)):
                            for q in range(4):
                                for kk in range(4):
                                    kt = 4 * kk + q
                                    so = q * 128 + kk * 32
                                    first = pi == 0 and q == 0 and kk == 0
                                    last = pi == 1 and q == 3 and kk == 3
                                    for hc in range(HC):
                                        nc.tensor.matmul(
                                            r_ps[hc * 32:(hc + 1) * 32, :],
                                            spk_t[cur][:, so:so + 32],
                                            A_h[:, kt * N_HID + hc * 512: kt * N_HID + hc * 512 + 512],
                                            start=first,
                                            stop=last,
                                            tile_position=(0, hc * 32),
                                            skip_group_check=True,
                                        )
                        t1t = fs.tile([128, 512], F32, tag="t1", bufs=1)
                        t1f = t1t[:]

                    y_hm = fs.tile([128, 512], F32, tag="yhm", bufs=1)
                    # per-block pipeline: add-C, transpose, tanh (from PSUM),
                    # threshold -- each 128-wide block flows independently so
                    # the next frame's matmuls can start on early blocks.
                    for j in range(4):
                        blk = slice(j * 128, (j + 1) * 128)
                        if t > 0:
                            nc.vector.tensor_add(t1f[:, blk], r_ps[:, blk], c_q[:, blk])
                        tp = tps.tile([128, 128], F32, tag="tp", name="tp")
                        nc.tensor.transpose(tp[:], t1f[:, blk], eye_sb[:])
                        nc.scalar.activation(
                            y_hm[:, blk], tp[:], mybir.ActivationFunctionType.Tanh
                        )
                        nc.vector.tensor_tensor(
                            spk_t[nxt][:, blk], y_hm[:, blk], tmh_sb[:, blk],
                            op=mybir.AluOpType.is_gt,
                        )

                    # off the spike chain: mem' = y + h, then next h and 0.5-h
                    # (h updated in place once the old value is consumed)
                    nc.vector.tensor_add(mem_t[nxt][:], y_hm[:], h_sb[:])
                    if t + 1 < frames:
                        nc.vector.tensor_scalar(
                            sph_sb[:], mem_t[nxt][:], THR, 0.5,
                            mybir.AluOpType.is_gt, mybir.AluOpType.mult,
                        )
                        nc.vector.tensor_scalar(
                            h_sb[:], mem_t[nxt][:], 0.5, -0.5,
                            mybir.AluOpType.mult, mybir.AluOpType.add,
                        )
                        nc.vector.tensor_add(h_sb[:], h_sb[:], sph_sb[:])
                        nc.vector.tensor_scalar(
                            tmh_sb[:], h_sb[:], -1.0, 0.5,
                            mybir.AluOpType.mult, mybir.AluOpType.add,
                        )

                    # outputs ride the scalar queue so they never delay the
                    # c_q / x reads on the sync queue
                    nc.scalar.dma_start(memsT[t], mem_t[nxt][:])
                    nc.scalar.dma_start(spikesT[t], spk_t[nxt][:])

                    # U chunk AFTER the frame's ops: its matmuls fill the PE
                    # during the frame tail, and its csts queue behind (not
                    # ahead of) this frame's tanhs on the ACT engine.
                    if t % 4 == 0 and t // 4 + PRE < btc_used:
                        emit_uchunk(t // 4 + PRE)

    nc.compile()
    return nc


def _host_prep(x, W_in, A, bias, mem_init, frames):
    """Build per-core input maps (shared arrays computed once)."""
    x = np.ascontiguousarray(x, dtype=np.float32)
    W_in = np.asarray(W_in, dtype=np.float32)
    A = np.asarray(A, dtype=np.float32)
    bias = np.asarray(bias, dtype=np.float32)
    mem_init = np.asarray(mem_init, dtype=np.float32)

    import ml_dtypes

    Apf = (ALPHA * A).reshape(KT, 128, N_HID).transpose(1, 0, 2).reshape(128, KT * N_HID)
    Apf = np.ascontiguousarray(Apf)
    Aph = Apf.astype(ml_dtypes.bfloat16)
    Apl = (Apf - Aph.astype(np.float32)).astype(ml_dtypes.bfloat16)

    # W at scale 2^11; fp16 hi + residual for the fp8 passes.
    W_aug = np.zeros((KIN, N_HID), dtype=np.float32)
    W_aug[:N_IN] = (1.0 - ALPHA) * W_in
    W_aug[N_IN] = (1.0 - ALPHA) * bias
    # mybir float8e4 == ml_dtypes.float8_e4m3 (IEEE-style, max 240, has inf):
    # clip before cast so a tail value can never encode as inf.
    def e4m3(a):
        return np.clip(a, -224.0, 224.0).astype(ml_dtypes.float8_e4m3)

    Ws = W_aug * np.float32(2.0 ** 11)
    Wh = Ws.astype(np.float16)                       # p1 operand
    Wl = Ws - Wh.astype(np.float32)
    W8h = e4m3(Ws * np.float32(2.0 ** -3))           # W*2^8  (max ~128)
    W8l = e4m3(Wl * np.float32(2.0 ** 6))            # Wl*2^17 (max ~32)

    def pack_w(Wm):
        return np.ascontiguousarray(
            Wm.reshape(KC, 128, N_HID).transpose(1, 0, 2))

    Wph = np.ascontiguousarray(
        Wh.reshape(KC, 128, N_HID).transpose(1, 0, 2).reshape(128, KC * N_HID))
    Wp8h = pack_w(W8h)
    Wp8l = pack_w(W8l)

    eye = np.eye(128, dtype=np.float32)

    in_maps = []
    for i in range(NCORES):
        xs = x[i * B:(i + 1) * B, :frames]            # [B, frames, N_IN]
        xTc = np.zeros((KIN, B * frames), dtype=np.float32)
        # xT[n, t*B + b] = x[b, t, n]
        xTc[:N_IN] = xs.transpose(2, 1, 0).reshape(N_IN, frames * B)
        xTc[N_IN] = 1.0
        if frames < FRAMES:
            full = np.zeros((KIN, BT), dtype=np.float32)
            full[:, : B * frames] = xTc
            xTc = full
        def e4m3(a):
            return np.clip(a, -224.0, 224.0).astype(ml_dtypes.float8_e4m3)

        def pack_x(a):
            # [KIN, BT] -> [BTC, 128(p), KC, 128(j)] contiguous per chunk
            return np.ascontiguousarray(
                a.reshape(KC, 128, BTC, 128).transpose(2, 1, 0, 3))

        xss = xTc * np.float32(2.0 ** 11)             # x*2^11
        xTh16 = xss.astype(np.float16)
        xl = xss - xTh16.astype(np.float32)
        xT8h = e4m3(xTh16.astype(np.float32) * np.float32(2.0 ** -6))  # fp16(x)*2^5
        xT8l = e4m3(xl.astype(np.float16).astype(np.float32)
                    * np.float32(2.0 ** 3))           # fp16(xl)*2^14
        xTh16 = pack_x(xTh16).reshape(BTC, 128, KIN)
        xT8h = pack_x(xT8h)
        xT8l = pack_x(xT8l)
        ms = mem_init[i * B:(i + 1) * B]              # [B, N_HID]
        # hm packing: hm[p, q*128 + hc*32 + b] = mem[b, hc*512 + q*128 + p]
        m0 = ms.reshape(B, 4, 4, 128).transpose(3, 2, 1, 0).reshape(128, 512)
        in_maps.append(
            {
                "Aph": Aph,
                "Apl": Apl,
                "Wph": Wph,
                "Wp8h": Wp8h,
                "Wp8l": Wp8l,
                "xTh": np.ascontiguousarray(xTh16),
                "xT8h": np.ascontiguousarray(xT8h),
                "xT8l": np.ascontiguousarray(xT8l),
                "mem0": np.ascontiguousarray(m0),
                "eye": eye,
            }
        )
    return in_maps


def kernel(x, W_in, A, bias, mem_init):
    global LAST_RESULT
    frames = int(os.environ.get("ANNRC_FRAMES", FRAMES))

    if frames not in _NC_CACHE:
        _NC_CACHE[frames] = _build_nc(frames)
    nc = _NC_CACHE[frames]

    in_maps = _host_prep(x, W_in, A, bias, mem_init, frames)
    res = run_bass_kernel_spmd(nc, in_maps, core_ids=list(range(NCORES)))
    LAST_RESULT = res

    mems = np.empty((BATCH, frames, N_HID), dtype=np.float32)
    spikes = np.empty((BATCH, frames, N_HID), dtype=np.float32)
    for i in range(NCORES):
        out = res.results[i]
        mt = out["memsT"].reshape(frames, 128, 4, 4, B).transpose(4, 0, 3, 2, 1)
        mems[i * B:(i + 1) * B] = mt.reshape(B, frames, N_HID)
        sp = np.asarray(out["spikesT"], np.float32).reshape(frames, 128, 4, 4, B)
        spikes[i * B:(i + 1) * B] = sp.transpose(4, 0, 3, 2, 1).reshape(B, frames, N_HID)
    return mems, spikes
